# revision 1
# baseline (speedup 1.0000x reference)
"""Trainium2 Bass kernel for nn_Net_34763465294339.

Four single-channel VALID convs (K=25/49/97/193, 16 output channels each) on
x[16,1,256,256], each squared + spatially averaged / scale -> stack -> fold
16 channels into 8 by adding halves. Output [16,8,4] f32.

Sharding: data-parallel over batch, 2 images per core, weights replicated.

Conv-as-matmul (output-stationary):
  PSUM tile per 8-output-row block: partitions m=(s,o)=8x16=128, free
  n=(j,b)=2*S (both images column-interleaved). Contraction k=(t,dj) over T kernel
  rows (T*K<=128; K=193 splits dj into 2 chunks). Accumulate over base-row
  sweep q=0..Q-1 (r0=i0+q*T) in PSUM.

  All per-q weight matrices are AP-offset slices of one padded matrix per
  chunk: M[(t,dj),(u,o)] = w[o, t+qmaxT-u, dj] (zero outside [0,K)), with
  lhsT_q = M[:, u0:u0+8, :] at u0=qmaxT-q*T.

  rhs tiles are shifted-row im2col tiles DMA'd straight from DRAM with
  overlapping-read APs, rotating through a small pool (re-fetched per group).

  Post: per block, DVE tensor_tensor_reduce (square, scaled, free-dim sum)
  into a stage column; a tiny fp32 fold-matmul (ones matrix) folds the
  (s,o)->o%8 partitions; per-(conv,image) column reduce; one DMA out.
"""
import os

import numpy as np
import ml_dtypes

import concourse.bass as bass
import concourse.bacc as bacc
import concourse.mybir as mybir
from concourse.tile import TileContext
from concourse.bass_utils import run_bass_kernel_spmd

BF16 = mybir.dt.bfloat16
F32 = mybir.dt.float32

IMG = 256
NCORES = 8
BLOCK_I = 8  # output rows per psum block
GROUP = 8    # psum blocks in flight (8 PSUM banks)

# (K, T, scale)
CONVS = [(25, 4, 1.0), (49, 2, 2.0), (97, 1, 4.0), (193, 1, 8.0)]
# rhs rotating-pool bufs per conv tag (>= max tiles in flight + prefetch)
RHS_BUFS = {25: 24, 49: 40, 97: 72, 193: 72}


def _conv_cfg(K, T):
    S = IMG - K + 1
    Q = (K + 7) // T
    U = (Q - 1) * T + 8
    chunks = [(0, K)] if T * K <= 128 else [(0, 128), (128, K)]
    return S, Q, U, chunks


def _build_M(w, K, T, scale):
    """w: [16,K,K] fp32, pre-scaled by sqrt(1/(S^2*scale)) so the squared
    conv outputs sum directly to the scaled mean. Returns fp32 [T*Kc, U*16]
    per dj-chunk."""
    S, Q, U, chunks = _conv_cfg(K, T)
    w = w * np.sqrt(1.0 / (float(S) * S * scale), dtype=np.float32)
    qmaxT = (Q - 1) * T
    out = []
    for (lo, hi) in chunks:
        Kc = hi - lo
        M = np.zeros((T * Kc, U, 16), dtype=np.float32)
        for t in range(T):
            for u in range(U):
                di = t + qmaxT - u
                if 0 <= di < K:
                    M[t * Kc:(t + 1) * Kc, u, :] = w[:, di, lo:hi].T
        out.append(np.ascontiguousarray(M.reshape(T * Kc, U * 16)))
    return out


def _build_fold():
    F = np.zeros((128, 8), dtype=np.float32)
    for p in range(128):
        F[p, (p % 16) % 8] = 1.0
    return F


def _col_layout(convs):
    col_base = {}
    c = 0
    for (K, T, scale) in convs:
        nb = (IMG - K + 1) // BLOCK_I
        for b in range(2):
            col_base[(K, b)] = c
            c += nb
    return col_base, c


def _build_nc(convs):
    nc = bacc.Bacc("TRN2", target_bir_lowering=False)
    x = nc.dram_tensor("x", [IMG, IMG, 2], BF16, kind="ExternalInput")
    m_handles = {}
    for (K, T, scale) in convs:
        S, Q, U, chunks = _conv_cfg(K, T)
        for idx, (lo, hi) in enumerate(chunks):
            Kc = hi - lo
            m_handles[(K, idx)] = nc.dram_tensor(
                f"m{K}_{idx}", [T * Kc, U * 16], BF16, kind="ExternalInput")
    fold = nc.dram_tensor("fold", [128, 8], F32, kind="ExternalInput")
    out = nc.dram_tensor("out", [2, 8, 4], F32, kind="ExternalOutput")

    col_base, TOT = _col_layout(convs)

    with TileContext(nc) as tc:
        with tc.tile_pool(name="consts", bufs=1) as cpool, \
             tc.tile_pool(name="rhsp", bufs=2) as rpool, \
             tc.tile_pool(name="scrp", bufs=4) as spool, \
             tc.tile_pool(name="accp", bufs=8, space="PSUM") as ppool:
            m_sb = {}
            for (K, idx), h in m_handles.items():
                mt = cpool.tile(list(h.shape), BF16, name=f"msb{K}_{idx}",
                                tag=f"m{K}_{idx}")
                nc.sync.dma_start(out=mt[:], in_=h[:])
                m_sb[(K, idx)] = mt
            fold_sb = cpool.tile([128, 8], F32, name="fold_sb", tag="fold")
            nc.sync.dma_start(out=fold_sb[:], in_=fold[:])
            stage = cpool.tile([128, TOT], F32, name="stage", tag="stage")

            for (K, T, scale) in convs:
                S, Q, U, chunks = _conv_cfg(K, T)
                qmaxT = (Q - 1) * T
                nb = S // BLOCK_I
                n = 2 * S
                nchunks = len(chunks)
                for g0 in range(0, nb, GROUP):
                    gblocks = list(range(g0, min(g0 + GROUP, nb)))
                    tiles = {}
                    psums = {}
                    for blk in gblocks:
                        psums[blk] = ppool.tile([128, n], F32,
                                                name=f"ps{K}_{blk}", tag="acc")
                    for q in range(Q):
                        u0 = qmaxT - q * T
                        for ci_, (lo, hi) in enumerate(chunks):
                            Kc = hi - lo
                            for blk in gblocks:
                                r0 = blk * BLOCK_I + q * T
                                key = (r0, ci_)
                                rt = tiles.get(key)
                                if rt is None:
                                    rt = rpool.tile(
                                        [T * Kc, n], BF16,
                                        name=f"r{K}_{r0}_{ci_}",
                                        tag=f"rhs{K}_{ci_}", bufs=RHS_BUFS[K])
                                    src = bass.AP(
                                        x, (r0 * IMG + lo) * 2,
                                        [[IMG * 2, T], [2, Kc], [1, n]])
                                    nc.gpsimd.dma_start(out=rt[:], in_=src)
                                    tiles[key] = rt
                                lhsT = m_sb[(K, ci_)].rearrange(
                                    "k (u o) -> k u o", o=16)[:, u0:u0 + 8, :]
                                nc.tensor.matmul(
                                    psums[blk][:], lhsT, rt[:],
                                    start=(q == 0 and ci_ == 0),
                                    stop=(q == Q - 1 and ci_ == nchunks - 1))
                    for blk in gblocks:
                        for b in range(2):
                            scr = spool.tile([128, S], F32,
                                             name=f"sq{K}_{blk}_{b}", tag="scr")
                            col = col_base[(K, b)] + blk
                            nc.scalar.activation(
                                out=scr[:],
                                in_=psums[blk][:, b::2],
                                func=mybir.ActivationFunctionType.Square,
                                accum_out=stage[:, col:col + 1])

            fold_ps = ppool.tile([8, TOT], F32, name="fold_ps", tag="acc")
            nc.tensor.matmul(fold_ps[:], fold_sb[:], stage[:],
                             start=True, stop=True)
            res = spool.tile([8, 8], F32, name="res", tag="res", bufs=1)
            for ci, (K, T, scale) in enumerate(CONVS):
                if (K, T, scale) not in convs:
                    continue
                nb = (IMG - K + 1) // BLOCK_I
                for b in range(2):
                    c0 = col_base[(K, b)]
                    oc = b * 4 + ci
                    nc.vector.reduce_sum(out=res[:8, oc:oc + 1],
                                         in_=fold_ps[:8, c0:c0 + nb],
                                         axis=mybir.AxisListType.X)
            dst = bass.AP(out, 0, [[4, 8], [32, 2], [1, 4]])
            nc.sync.dma_start(out=dst, in_=res[:8, :])
    return nc


_NC_CACHE = {}


def _get_nc(convs_key):
    if convs_key not in _NC_CACHE:
        nc = _build_nc(list(convs_key))
        nc.compile()
        _NC_CACHE[convs_key] = nc
    return _NC_CACHE[convs_key]


def kernel(x, w0, w1, w2, w3, _convs=None, _trace=False, _tmpdir=None):
    convs = CONVS if _convs is None else _convs
    ws = {25: w0, 49: w1, 97: w2, 193: w3}

    x = np.asarray(x, dtype=np.float32).reshape(16, IMG, IMG)
    xb = x.astype(ml_dtypes.bfloat16)

    shared = {}
    for (K, T, scale) in convs:
        w = np.asarray(ws[K], dtype=np.float32).reshape(16, K, K)
        for idx, M in enumerate(_build_M(w, K, T, scale)):
            shared[f"m{K}_{idx}"] = M.astype(ml_dtypes.bfloat16)
    shared["fold"] = _build_fold()

    in_maps = []
    for c in range(NCORES):
        m = dict(shared)
        m["x"] = np.ascontiguousarray(xb[2 * c:2 * c + 2].transpose(1, 2, 0))
        in_maps.append(m)

    nc = _get_nc(tuple(convs))
    kw = {}
    if _trace:
        kw.update(trace=True, tmpdir=_tmpdir)
    r = run_bass_kernel_spmd(nc, in_maps, list(range(NCORES)), **kw)
    out = np.concatenate([np.asarray(r.results[c]["out"], dtype=np.float32)
                          for c in range(NCORES)], axis=0)
    if _trace:
        kernel.last_exec_time_ns = r.exec_time_ns
        kernel.last_results = r
    return out



# revision 56
# speedup vs baseline: 9.8766x; 9.8766x over previous
"""Trainium2 Bass kernel for nn_Net_34763465294339.

Four single-channel VALID convs (K=25/49/97/193, 16 output channels each) on
x[16,1,256,256], each squared + spatially averaged / scale -> stack -> fold
16 channels into 8 by adding halves. Output [16,8,4] f32.

Sharding: data-parallel over batch, 2 images per core, weights replicated.

Conv-as-matmul (output-stationary), fp8 DoubleRow:
  PSUM tile per 8-output-row block: partitions m=(u,o)=8x16=128, free
  n=(j,b)=2*S (both images column-interleaved). Contraction k=(t,dj) over T
  kernel rows (T*K<=128). Base-row sweep q (r0=8*blk+T*q) accumulates in
  PSUM; consecutive q's are executed two-at-a-time as the two lanes of an
  fp8 DoubleRow matmul (lhsT [k,2,m], rhs [k,2,n], 0.5 cycles/row).

  rhs: the im2col tile for base row r0 is slot s=r0/step (step=gcd(8,T)) of
  one logical supertile whose slot s holds the shifted window x[step*s+t,
  j+dj]. Supertiles live in SBUF as 8-slot tiles loaded once by 3D-AP DMAs
  (round-robined over sync/vector/gpsimd queues); a DoubleRow rhs is just a
  2-slot free-dim slice. K=193 instead uses its two dj-chunks (0:128,
  128:193 zero-padded to 256) as the DoubleRow lanes, with slots laid out
  (slot, lane, jb); x gets one extra zero row so lane-1 reads stay in
  bounds. Weights for the pad are zero.

  Weights are host-built fp8 matrices pre-scaled by a per-conv power of two
  (dynamic-range fit); the energy normalization 1/(S^2*scale*2^2b) is
  applied on host to the tiny [2,8,4] device output.

  Post: per block, scalar-engine Square activation with free-dim accumulate
  into a stage column; fp32 fold-matmul adds channel o and o+8; per-
  (conv,image) column reduce; one DMA out.
"""
import os

import numpy as np
import ml_dtypes

import concourse.bass as bass
import concourse.bacc as bacc
import concourse.mybir as mybir
from concourse.tile import TileContext
from concourse.bass_utils import run_bass_kernel_spmd

FP8 = mybir.dt.float8e4
SCR_DT = mybir.dt.bfloat16
F32 = mybir.dt.float32
NP_FP8 = mybir.dt.np(mybir.dt.float8e4)

IMG = 256
NCORES = 8
BLOCK_I = 8   # output rows per psum block
TILE_SLOTS = 8

# (K, T, scale)
CONVS = [(25, 4, 1.0), (49, 2, 2.0), (97, 1, 4.0), (193, 1, 8.0)]
EXEC_ORDER = [97, 193, 49, 25]
GROUPS = {25: 4, 49: 4, 97: 4, 193: 4}
ST_BUFS = {25: 32, 49: 64, 97: 18, 193: 10}
TSLOTS = {25: 2, 49: 2, 97: 8, 193: 8}
FP8_TARGET = 192.0  # max |w|*2^b after scaling (e4m3 max finite 240)


def _conv_cfg(K, T):
    S = IMG - K + 1
    Q = (K + 7) // T
    U = (Q - 1) * T + 8
    return S, Q, U


def _wexp(w):
    """Power-of-two exponent b with max|w*2^b| <= FP8_TARGET."""
    m = float(np.max(np.abs(w)))
    return int(np.floor(np.log2(FP8_TARGET / m)))


def _build_M(w, K, T):
    """w: [16,K,K] fp32 (already 2^b-scaled). Returns fp32 M [T*K, U, 16]
    with M[(t,dj),u,:] = w[:, t+qmaxT-u, dj] (zero outside [0,K))."""
    S, Q, U = _conv_cfg(K, T)
    qmaxT = (Q - 1) * T
    M = np.zeros((T * K, U, 16), dtype=np.float32)
    for t in range(T):
        for u in range(U):
            di = t + qmaxT - u
            if 0 <= di < K:
                M[t * K:(t + 1) * K, u, :] = w[:, di, :].T
    return M


def _build_M2(w, K, T):
    """Paired DoubleRow weights [T*K, Q//2, 2, 8, 16] fp32."""
    S, Q, U = _conv_cfg(K, T)
    qmaxT = (Q - 1) * T
    M = _build_M(w, K, T)
    M2 = np.zeros((T * K, Q // 2, 2, 8, 16), dtype=np.float32)
    for p in range(Q // 2):
        for i in range(2):
            u0 = qmaxT - (2 * p + i) * T
            M2[:, p, i, :, :] = M[:, u0:u0 + 8, :]
    return np.ascontiguousarray(M2.reshape(T * K, Q // 2 * 256))


def _build_M193(w):
    """Chunk-lane weights for K=193: [128, 2, U, 16] fp32 with
    lane i, row k -> w[:, di, i*128+k] (zero for i=1, k>=65)."""
    K, T = 193, 1
    S, Q, U = _conv_cfg(K, T)
    qmax = Q - 1
    M = np.zeros((128, 2, U, 16), dtype=np.float32)
    for i in range(2):
        kc = 128 if i == 0 else K - 128
        for u in range(U):
            di = qmax - u
            if 0 <= di < K:
                M[:kc, i, u, :] = w[:, di, i * 128:i * 128 + kc].T
    return np.ascontiguousarray(M.reshape(128, 2 * U * 16))


def _build_fold():
    F = np.zeros((128, 8), dtype=np.float32)
    for p in range(128):
        F[p, (p % 16) % 8] = 1.0
    return F


def _col_layout(convs):
    col_base = {}
    c = 0
    for (K, T, scale) in convs:
        nb = (IMG - K + 1) // BLOCK_I
        for b in range(2):
            col_base[(K, b)] = c
            c += nb
    return col_base, c


def _nslots(K, T, step):
    S, Q, U = _conv_cfg(K, T)
    nb = S // BLOCK_I
    return (8 * (nb - 1) + T * (Q - 1)) // step + 1


def _build_nc(convs):
    nc = bacc.Bacc("TRN2", target_bir_lowering=False)
    # one extra zero row so K=193 lane-1 overreads stay in bounds
    x = nc.dram_tensor("x", [IMG + 1, IMG, 2], FP8, kind="ExternalInput")
    m_handles = {}
    for (K, T, scale) in convs:
        S, Q, U = _conv_cfg(K, T)
        if K == 193:
            m_handles[K] = nc.dram_tensor("m193", [128, 2 * U * 16], FP8,
                                          kind="ExternalInput")
        else:
            m_handles[K] = nc.dram_tensor(f"m{K}", [T * K, Q // 2 * 256], FP8,
                                          kind="ExternalInput")
    fold = nc.dram_tensor("fold", [128, 8], F32, kind="ExternalInput")
    out = nc.dram_tensor("out", [2, 8, 4], F32, kind="ExternalOutput")

    col_base, TOT = _col_layout(convs)
    nq = int(os.environ.get("K_NDMAQ", "2"))
    dmaeng = [nc.sync, nc.gpsimd, nc.scalar][:nq]

    with TileContext(nc) as tc:
        with tc.tile_pool(name="consts", bufs=1) as cpool, \
             tc.tile_pool(name="stp", bufs=2) as stpool, \
             tc.tile_pool(name="scrp", bufs=8) as spool, \
             tc.tile_pool(name="accp", bufs=8, space="PSUM") as ppool:
            fold_sb = cpool.tile([128, 8], F32, name="fold_sb", tag="fold")
            stage = cpool.tile([128, TOT], F32, name="stage", tag="stage")

            # per-conv state
            st = {}
            for (K, T, scale) in convs:
                S, Q, U = _conv_cfg(K, T)
                nb = S // BLOCK_I
                step = 4 if T == 4 else (2 if T == 2 else 1)
                ns = _nslots(K, T, step)
                tsl = TSLOTS[K]
                starts = []
                p = 0
                while p < ns:
                    starts.append(p)
                    p += tsl
                bounds = [(st0, min(en, ns) - st0) for st0, en in
                          zip(starts, starts[1:] + [ns])]
                s2t = []
                for ti, (st0, cnt) in enumerate(bounds):
                    s2t += [(ti, sl - st0) for sl in range(st0, st0 + cnt)]
                st[K] = dict(
                    T=T, S=S, Q=Q, qmaxT=(Q - 1) * T, nb=nb, n=2 * S,
                    step=step, ns=ns, tslots=tsl, tbounds=bounds, s2t=s2t,
                    lane193=(K == 193), tiles={}, m_sb=None, mm=None)
                st[K]["slot_w"] = 2 * st[K]["n"] if st[K]["lane193"] \
                    else st[K]["n"]

            dmai = [0]

            def issue_m(K, pin=None):
                c = st[K]
                if c["m_sb"] is not None:
                    return
                h = m_handles[K]
                m_sb = cpool.tile(list(h.shape), FP8, name=f"msb{K}",
                                  tag=f"m{K}")
                # tiny first chunk so the first pair's weights land fast,
                # remainder in larger chunks across queues
                fsz = h.shape[1]
                bounds = [0, min(512, fsz)]
                stepw = 1024
                while bounds[-1] < fsz:
                    bounds.append(min(bounds[-1] + stepw, fsz))
                    stepw = min(stepw * 2, 8192)
                for ci0, ce in zip(bounds, bounds[1:]):
                    if pin is not None:
                        eng = dmaeng[pin]
                    else:
                        eng = dmaeng[dmai[0] % len(dmaeng)]
                        dmai[0] += 1
                    eng.dma_start(out=m_sb[:, ci0:ce], in_=h[:, ci0:ce])
                c["m_sb"] = m_sb
                if c["lane193"]:
                    c["mm"] = m_sb.rearrange("k (i u o) -> k i u o",
                                             i=2, o=16)
                else:
                    c["mm"] = m_sb.rearrange("k (p i u o) -> k p i u o",
                                             i=2, u=8, o=16)

            def issue_tile(K, it, pin=None):
                c = st[K]
                if it in c["tiles"]:
                    return c["tiles"][it]
                if pin is not None:
                    engs = [dmaeng[pin]]
                else:
                    engs = None
                T, n, step, ns = c["T"], c["n"], c["step"], c["ns"]
                slot_w = c["slot_w"]
                s0, cnt = c["tbounds"][it]
                kpart = 128 if c["lane193"] else T * K
                pt = stpool.tile(
                    [kpart, cnt * slot_w], FP8,
                    name=f"st{K}_{it}", tag=f"st{K}", bufs=ST_BUFS[K])
                def _eng():
                    if engs is not None:
                        return engs[0]
                    e = dmaeng[dmai[0] % len(dmaeng)]
                    dmai[0] += 1
                    return e

                if c["lane193"]:
                    ptv = pt.rearrange("k (c i n) -> k c i n", i=2, n=n)
                    for lane in range(2):
                        src = bass.AP(
                            x, s0 * IMG * 2 + lane * 128 * 2,
                            [[2, 128], [IMG * 2, cnt], [1, n]])
                        _eng().dma_start(out=ptv[:, :, lane, :], in_=src)
                elif T == 1:
                    # partition dim is just dj: whole tile in one 3D DMA
                    src = bass.AP(
                        x, s0 * step * IMG * 2,
                        [[2, K], [step * IMG * 2, cnt], [1, n]])
                    _eng().dma_start(out=pt[:], in_=src)
                else:
                    # per-slot 3D-AP DMAs (t,dj on partitions, jb free)
                    for sl in range(cnt):
                        src = bass.AP(
                            x, (s0 + sl) * step * IMG * 2,
                            [[IMG * 2, T], [2, K], [1, n]])
                        _eng().dma_start(out=pt[:, sl * n:(sl + 1) * n],
                                         in_=src)
                c["tiles"][it] = pt
                return pt

            def emit_group(K, g0, sz=None):
                c = st[K]
                T, S, Q, n = c["T"], c["S"], c["Q"], c["n"]
                qmaxT, nb, step = c["qmaxT"], c["nb"], c["step"]
                tslots, slot_w = c["tslots"], c["slot_w"]
                mm = c["mm"]
                npair = Q if c["lane193"] else Q // 2
                gblocks = list(range(g0, min(g0 + (sz or GROUPS[K]), nb)))
                # ensure tiles for this group are issued (slot order)
                s2t = c["s2t"]
                needed = sorted({
                    s2t[(8 * blk + T * q) // step][0]
                    for blk in gblocks for q in range(Q)})
                for it in needed:
                    issue_tile(K, it)
                ptag = os.environ.get("K_PTAG_SPLIT") and ("accB" if K in (97, 193) else "accF") or "acc"
                psums = {}
                for blk in gblocks:
                    psums[blk] = ppool.tile([128, n], F32,
                                            name=f"ps{K}_{blk}", tag=ptag,
                                            bufs=int(os.environ.get("K_PBUFS", "8")))
                for p in range(npair):
                    if c["lane193"]:
                        u0 = qmaxT - p
                        lhsT = mm[:, :, u0:u0 + 8, :]
                    else:
                        lhsT = mm[:, p, :, :, :]
                    for blk in gblocks:
                        q = p if c["lane193"] else 2 * p
                        s = (8 * blk + T * q) // step
                        ti, soff = s2t[s]
                        pt = c["tiles"][ti]
                        off = soff * slot_w
                        rhs = pt[:, off:off + 2 * n].rearrange(
                            "k (i n) -> k i n", i=2)
                        nc.tensor.matmul(
                            psums[blk][:], lhsT, rhs,
                            start=(p == 0), stop=(p == npair - 1),
                            perf_mode=mybir.MatmulPerfMode.DoubleRow)
                for blk in gblocks:
                    scr = spool.tile([128, n], SCR_DT,
                                     name=f"sq{K}_{blk}", tag="scr")
                    nc.scalar.activation(
                        out=scr[:],
                        in_=psums[blk][:],
                        func=mybir.ActivationFunctionType.Square)
                    for b in range(2):
                        col = col_base[(K, b)] + blk
                        nc.vector.reduce_sum(out=stage[:, col:col + 1],
                                             in_=scr[:, b::2],
                                             axis=mybir.AxisListType.X)

            # schedule: big convs (97, 193) carry the PE load; small convs'
            # groups are interleaved between them so their activations drain
            # under the big convs' matmul streams.
            have = {K for (K, T, s) in convs}
            border = [int(t) for t in
                      os.environ.get("K_BIGORDER", "193,97").split(",")]
            biglist = [K for K in border if K in have]
            filllist = [K for K in (49, 25) if K in have]
            bigs = []
            for K in biglist:
                nb = st[K]["nb"]
                g0 = 0
                while g0 < nb:
                    bigs.append((K, (g0, GROUPS[K])))
                    g0 += GROUPS[K]
            fills = []
            remaining = {}
            for K in filllist:
                nb = st[K]["nb"]
                gs = [(g0, min(GROUPS[K], nb - g0))
                      for g0 in range(0, nb, GROUPS[K])]
                if gs and gs[-1][1] >= 4:  # ramp-out: short final drain
                    g0, szl = gs.pop()
                    gs += [(g0, 2), (g0 + 2, szl - 2)]
                remaining[K] = gs
            while any(remaining.values()):
                for K in filllist[::-1]:
                    if remaining[K]:
                        g0, szf = remaining[K].pop(0)
                        fills.append((K, g0, szf))
            others = [K for K in have if K not in biglist + filllist]
            for K in others:
                issue_m(K)
                for g0 in range(0, st[K]["nb"], GROUPS[K]):
                    emit_group(K, g0)
            nfill = len(fills)
            nbig = len(bigs)
            fi = 0
            nointer = os.environ.get("K_NOINTER", "1") != "0"
            ninter193 = int(os.environ.get("K_INTER193", "0"))
            if bigs:
                K0 = bigs[0][0]
                h0 = m_handles[K0]
                c0w = min(512, h0.shape[1])
                m_sb0 = cpool.tile(list(h0.shape), FP8, name=f"msb{K0}",
                                   tag=f"m{K0}")
                nc.sync.dma_start(out=m_sb0[:, 0:c0w], in_=h0[:, 0:c0w])
                issue_tile(K0, 0, pin=1)
                issue_tile(K0, 1, pin=0)
                # remaining weight chunks, escalating sizes, across queues
                bnds = [c0w]
                stepw = 1024
                while bnds[-1] < h0.shape[1]:
                    bnds.append(min(bnds[-1] + stepw, h0.shape[1]))
                    stepw = min(stepw * 2, 8192)
                for i, (a, b) in enumerate(zip(bnds, bnds[1:])):
                    dmaeng[i % 2].dma_start(out=m_sb0[:, a:b],
                                            in_=h0[:, a:b])
                st[K0]["m_sb"] = m_sb0
                if st[K0]["lane193"]:
                    st[K0]["mm"] = m_sb0.rearrange("k (i u o) -> k i u o",
                                                   i=2, o=16)
                else:
                    st[K0]["mm"] = m_sb0.rearrange(
                        "k (p i u o) -> k p i u o", i=2, u=8, o=16)
            for bi, (K, gsz) in enumerate(bigs):
                issue_m(K)
                # stagger remaining const loads, one conv per big group
                defer = biglist[1:] + filllist
                if 1 <= bi <= len(defer):
                    issue_m(defer[bi - 1])
                emit_group(K, gsz[0], gsz[1])
                if nointer:
                    want = ninter193 * max(0, bi - (nbig - 3)) if K == 193 \
                        else 0
                    want = fi + want if want else 0
                else:
                    want = (bi + 1) * nfill // nbig
                while fi < min(want, nfill):
                    issue_m(fills[fi][0])
                    emit_group(fills[fi][0], fills[fi][1], fills[fi][2])
                    fi += 1
            while fi < nfill:
                issue_m(fills[fi][0])
                emit_group(fills[fi][0], fills[fi][1], fills[fi][2])
                fi += 1

            # per-conv folds (emitted big->fill order so only the last
            # conv's activations gate the output chain)
            nc.gpsimd.dma_start(out=fold_sb[:], in_=fold[:])
            fold_ps = ppool.tile([8, TOT], F32, name="fold_ps",
                                 tag=os.environ.get("K_PTAG_SPLIT") and "accB" or "acc",
                                 bufs=int(os.environ.get("K_PBUFS", "8")))
            res = spool.tile([8, 8], F32, name="res", tag="res", bufs=1)
            emit_order = [t[0] for t in bigs] + [t[0] for t in fills]
            seen = []
            for K in emit_order + [K for K in have]:
                if K in seen or K not in have:
                    continue
                seen.append(K)
                ci = [i for i, (Kc, T, s) in enumerate(CONVS) if Kc == K][0]
                nb = (IMG - K + 1) // BLOCK_I
                c0 = col_base[(K, 0)]
                nc.tensor.matmul(fold_ps[:8, c0:c0 + 2 * nb], fold_sb[:],
                                 stage[:, c0:c0 + 2 * nb],
                                 start=True, stop=True)
                for b in range(2):
                    cb = col_base[(K, b)]
                    oc = b * 4 + ci
                    nc.vector.reduce_sum(out=res[:8, oc:oc + 1],
                                         in_=fold_ps[:8, cb:cb + nb],
                                         axis=mybir.AxisListType.X)
            dst = bass.AP(out, 0, [[4, 8], [32, 2], [1, 4]])
            nc.sync.dma_start(out=dst, in_=res[:8, :])
    return nc


_NC_CACHE = {}


def _get_nc(convs_key):
    if convs_key not in _NC_CACHE:
        nc = _build_nc(list(convs_key))
        nc.compile()
        _NC_CACHE[convs_key] = nc
    return _NC_CACHE[convs_key]


def build_in_maps(x, w0, w1, w2, w3, convs=None):
    """Host prep: returns (in_maps, post_scale[4])."""
    convs = CONVS if convs is None else convs
    ws = {25: w0, 49: w1, 97: w2, 193: w3}

    x = np.asarray(x, dtype=np.float32).reshape(16, IMG, IMG)
    xp = np.zeros((16, IMG + 1, IMG), dtype=np.float32)
    xp[:, :IMG, :] = x
    x8 = xp.astype(NP_FP8)

    shared = {}
    post = np.ones(4, dtype=np.float64)
    for ci, (K, T, scale) in enumerate(CONVS):
        if (K, T, scale) not in convs:
            continue
        S, Q, U = _conv_cfg(K, T)
        w = np.asarray(ws[K], dtype=np.float32).reshape(16, K, K)
        b = _wexp(w)
        wscaled = w * np.float32(2.0 ** b)
        if K == 193:
            shared["m193"] = _build_M193(wscaled).astype(NP_FP8)
        else:
            shared[f"m{K}"] = _build_M2(wscaled, K, T).astype(NP_FP8)
        post[ci] = 1.0 / (4.0 ** b * float(S) * S * scale)
    shared["fold"] = _build_fold()

    in_maps = []
    for c in range(NCORES):
        m = dict(shared)
        m["x"] = np.ascontiguousarray(
            x8[2 * c:2 * c + 2].transpose(1, 2, 0))
        in_maps.append(m)
    return in_maps, post


def kernel(x, w0, w1, w2, w3, _convs=None, _trace=False, _tmpdir=None):
    convs = CONVS if _convs is None else _convs
    in_maps, post = build_in_maps(x, w0, w1, w2, w3, convs)

    nc = _get_nc(tuple(convs))
    kw = {}
    if _trace:
        kw.update(trace=True, tmpdir=_tmpdir)
    r = run_bass_kernel_spmd(nc, in_maps, list(range(NCORES)), **kw)
    out = np.concatenate([np.asarray(r.results[c]["out"], dtype=np.float32)
                          for c in range(NCORES)], axis=0)
    out = (out * post[None, None, :]).astype(np.float32)
    if _trace:
        kernel.last_exec_time_ns = r.exec_time_ns
        kernel.last_results = r
    return out
